# revision 12
# baseline (speedup 1.0000x reference)
"""Multi-head attention kernel for Trainium2, sharded over 8 NeuronCores.

Problem: B=2, S=2048, D=1024, H=16 heads (DK=64).
  out = softmax(mask ? (XqWq^T)(XkWk^T)^T/8 : -1e9) (XvWv^T) Wo^T

Sharding: core c handles batch b=c//4 and 4 heads hg=c%4 (tensor parallel
over heads, data parallel over batch). Each core emits TWO partial
output projections (one per head-pair) y_hp = Wo_hp^T @ attn_hp [D, S];
the host sums the 8 bf16 partials per batch and transposes back.

Design: fully "transposed" on-chip layout (scores s_T[k, q], keys on
partitions) so PV consumes softmax probabilities directly as the moving
operand. Softmax denominators ride a ones-column appended to each head's
V (row 64 of the PV psum). The mask folds in multiplicatively post-exp
(exact: exp(-1e9)==0 in fp32, no all-masked rows); no max-subtraction
needed (scores are O(5)).

Schedule: one global 128-slot software pipeline, slot = (qh, hp, qc, kt)
with q processed 512 wide. Per slot: PE does scores(t+1) [two 64-row
tile_position matmuls, concurrent], PV(t-2), and one "drip" unit of
projection/output work; ACT does one [128,1024] exp; DVE does one
combined mask-multiply (broadcast AP) plus drip evictions. K/Q/V
projections and the output projection are chunked and dripped
just-in-time into the slot stream so the PE never idles (keeps the HAM
clock gate at 2.4 GHz) and no phase runs with idle ACT/DVE.
PSUM: scores [128,1024]x2 bufs (4 banks) + PV accum [65,512]x2 (2) +
drip [128,512]x2 (2) = 8 banks exactly.
"""

import sys

sys.path.insert(0, "/opt/trn_rl_repo")

import numpy as np
import ml_dtypes
from contextlib import ExitStack

B, S, D, H = 2, 2048, 1024, 16
DK = D // H  # 64
N_CORES = 8
HPC = H // (N_CORES // B)  # 4 heads per core
EPC = HPC * DK  # 256 head-dims per core
P = 128
BF16 = ml_dtypes.bfloat16

_CACHE = {}

NSLOT = 128  # (qh 2) x (hp 2) x (qc 2) x (kt 16)


def _slot(t):
    qh, r = divmod(t, 64)
    hp, r = divmod(r, 32)
    qc, kt = divmod(r, 16)
    return qh, hp, qc, kt


def _patch_tile_drain():
    """This walrus build allows only ONE sync-wait command on a Drain
    (CoreV3GenImpl setupSyncWait). Split the tail-drain waits across
    multiple drain instructions, one wait each."""
    import concourse.tile as tile
    from concourse import mybir
    from concourse.vector_clock import ScopedClock

    if getattr(tile.TileContext, "_drain_split_patch", False):
        return

    def _patched(self, tick_clock, wait_clock):
        nc = self.nc
        drain_inst = nc.sync.drain()
        wait_clock.add_sem_waits(
            drain_inst.ins, ScopedClock({None: tick_clock.global_clock})
        )
        si = drain_inst.ins.sync_info
        if si is not None and si.on_wait is not None and len(si.on_wait) > 1:
            extras = list(si.on_wait[1:])
            del si.on_wait[1:]
            for w in extras:
                d2 = nc.sync.drain()
                d2.ins.sync_info = mybir.SyncInfo(on_wait=[w], on_update=[])
        nc.all_engine_barrier()
        assert self.sems is not None
        popped = nc._tile_sem_poison_stack.pop()
        assert popped is self._sem_poison
        nc.clear_and_free_semaphores(list(self.sems.allocated().values()))
        nc.all_engine_barrier()

    tile.TileContext._drain_and_barrier = _patched
    tile.TileContext._drain_split_patch = True


def _split_multi_waits(nc):
    """This walrus build supports only ONE sync-wait command per
    instruction. Hoist extra waits onto preceding same-engine NoOps --
    engine sequencers process their stream in order, so a NoOp's wait
    gates everything after it on that engine."""
    from concourse import mybir

    ctr = [0]
    for fn in nc.m.functions:
        for blk in fn.blocks:
            insts = blk.instructions
            i = 0
            while i < len(insts):
                inst = insts[i]
                si = getattr(inst, "sync_info", None)
                waits = list(si.on_wait) if si is not None and si.on_wait else []
                if len(waits) > 1:
                    keep = waits[-1]
                    for w in waits[:-1]:
                        ctr[0] += 1
                        nop = mybir.InstNoOp(
                            name=f"wsplit_{ctr[0]}",
                            engine=inst.engine,
                            bass_nofuse=True,
                            sync_info=mybir.SyncInfo(on_wait=[w], on_update=[]),
                        )
                        insts.insert(i, nop)
                        i += 1
                    live = si.on_wait
                    del live[:-1]
                i += 1
    return ctr[0]


def _build_bass():
    import concourse.bass as bass
    import concourse.tile as tile
    from concourse import mybir

    _patch_tile_drain()
    bf16 = mybir.dt.bfloat16
    f32 = mybir.dt.float32
    Exp = mybir.ActivationFunctionType.Exp
    Copy = mybir.ActivationFunctionType.Copy
    Mult = mybir.AluOpType.mult

    nc = bass.Bass()
    xq = nc.dram_tensor("xq_t", [D, S], bf16, kind="ExternalInput")
    xk = nc.dram_tensor("xk_t", [D, S], bf16, kind="ExternalInput")
    xv = nc.dram_tensor("xv_t", [D, S], bf16, kind="ExternalInput")
    f8 = mybir.dt.float8e4
    mk = nc.dram_tensor("mask_t", [S, S], f8, kind="ExternalInput")
    wq = nc.dram_tensor("wq_t", [D, EPC], bf16, kind="ExternalInput")
    wk = nc.dram_tensor("wk_t", [D, EPC], bf16, kind="ExternalInput")
    wv = nc.dram_tensor("wv_t", [D, EPC], bf16, kind="ExternalInput")
    wo = nc.dram_tensor("wo_t", [EPC, D], bf16, kind="ExternalInput")
    ya = nc.dram_tensor("ya_t", [D, S], bf16, kind="ExternalOutput")
    yb = nc.dram_tensor("yb_t", [D, S], bf16, kind="ExternalOutput")
    y_dram = [ya, yb]
    # DRAM scratch for partition-broadcasting softmax denominators
    # (SBUF->SBUF DMA cannot broadcast across partitions; DRAM sources
    # can). rsum holds the raw sums row, read back as [128, 4] so the
    # reciprocal uses all DVE lanes; rrec holds the reciprocal for the
    # partition-broadcast read.
    rsum_dram = nc.dram_tensor("rsum_scratch", [16, 512], f32, kind="Internal")
    rrec_dram = nc.dram_tensor("rrec_scratch", [16, 512], f32, kind="Internal")
    fence_dram = nc.dram_tensor("fence_scratch", [4, 16], bf16, kind="Internal")

    VW = HPC * (DK + 1)  # 260: V columns + ones column per head

    with tile.TileContext(nc) as tc:
        with ExitStack() as ctx:
            xt_pool = ctx.enter_context(tc.tile_pool(name="xt", bufs=11))
            wt_pool = ctx.enter_context(tc.tile_pool(name="wt", bufs=3))
            wo_pool = ctx.enter_context(tc.tile_pool(name="wo", bufs=2))
            qk_pool = ctx.enter_context(tc.tile_pool(name="qk", bufs=4))
            v_pool = ctx.enter_context(tc.tile_pool(name="v", bufs=16))
            mask_pool = ctx.enter_context(tc.tile_pool(name="mask", bufs=16))
            et_pool = ctx.enter_context(tc.tile_pool(name="et", bufs=2))
            pt_pool = ctx.enter_context(tc.tile_pool(name="pt", bufs=3))
            out_pool = ctx.enter_context(tc.tile_pool(name="outsb", bufs=2))
            yev_pool = ctx.enter_context(tc.tile_pool(name="yev", bufs=3))
            otmp_pool = ctx.enter_context(tc.tile_pool(name="otmp", bufs=3))
            bc_pool = ctx.enter_context(tc.tile_pool(name="bc", bufs=3))
            r_pool = ctx.enter_context(tc.tile_pool(name="r", bufs=4))
            ps_s = ctx.enter_context(tc.tile_pool(name="ps_s", bufs=2, space="PSUM"))
            ps_o = ctx.enter_context(tc.tile_pool(name="ps_o", bufs=2, space="PSUM"))
            ps_d = ctx.enter_context(tc.tile_pool(name="ps_d", bufs=2, space="PSUM"))

            # ---------------- persistent SBUF tensors ----------------
            qt_sb = [qk_pool.tile([P, S], bf16, tag="qk", name="qt") for _ in range(2)]
            kt_sb = [qk_pool.tile([P, S], bf16, tag="qk", name="ktl") for _ in range(2)]
            out_sb = [out_pool.tile([P, S], bf16, tag="outsb", name="outsb")
                      for _ in range(2)]
            w_sb = {}
            wo_sb = []
            xc = {}      # (which, chunk) -> [128, 8, 512] tile
            v_sb = {}    # kt -> [128, 260]
            mask_sb = {}  # (qh, kt) -> [128, 1024]

            def dma_w(which, t, eng=None):
                wt = wt_pool.tile([P, 8, EPC], bf16, tag="wt", name="wt")
                (eng or nc.sync).dma_start(
                    wt[:], t[:, :].rearrange("(k p) e -> p k e", p=P))
                w_sb[which] = wt

            def dma_x(which, t, c, eng=None):
                x = xt_pool.tile([P, 8, 512], bf16, tag="xt", name="xt")
                (eng or nc.sync).dma_start(
                    x[:], t[:, c * 512:(c + 1) * 512].rearrange(
                        "(k p) s -> p k s", p=P))
                xc[(which, c)] = x

            def dma_wo():
                for k in range(2):
                    wt = wo_pool.tile([P, D], bf16, tag="wo", name="wo")
                    nc.sync.dma_start(wt[:], wo[k * P:(k + 1) * P, :])
                    wo_sb.append(wt)

            def dma_mask(qh, kt):
                # gpsimd software-DGE casts fp8 mask to bf16 in flight
                mt = mask_pool.tile([P, 1024], bf16, tag="mask", name="mask")
                mask_sb[(qh, kt)] = mt
                nc.gpsimd.dma_start(
                    mt[:], mk[kt * P:(kt + 1) * P,
                              qh * 1024:(qh + 1) * 1024])

            # ---------------- drip work units (PE + DVE evict) ---------
            def kq_half(dst_tiles, wname, xname, hp, c, h, act=True):
                ps = ps_d.tile([P, 512], f32, tag="d", name="psd")
                x = xc[(xname, c)]
                w = w_sb[wname]
                for k in range(8):
                    nc.tensor.matmul(
                        ps[:, 0:256],
                        lhsT=w[:, k, hp * P:(hp + 1) * P],
                        rhs=x[:, k, h * 256:(h + 1) * 256],
                        start=(k == 0), stop=(k == 7))
                dst = dst_tiles[hp][:, c * 512 + h * 256:c * 512 + (h + 1) * 256]
                if act:
                    nc.scalar.activation(dst, ps[:, 0:256], Copy)
                else:
                    nc.vector.tensor_copy(dst, ps[:, 0:256])

            def v_proj(kt):
                ps = ps_d.tile([P, 512], f32, tag="d", name="psd")
                x = xc[("xv", kt // 4)]
                w = w_sb["wv"]
                for k in range(8):
                    nc.tensor.matmul(
                        ps[:, 0:EPC],
                        lhsT=x[:, k, (kt % 4) * P:(kt % 4 + 1) * P],
                        rhs=w[:, k, :],
                        start=(k == 0), stop=(k == 7))
                vt = v_pool.tile([P, VW], bf16, tag="v", name="vt")
                vs = vt[:].rearrange("p (h x) -> p h x", h=HPC)
                nc.scalar.activation(
                    vs[:, :, 0:DK],
                    ps[:, 0:EPC].rearrange("p (h x) -> p h x", h=HPC), Copy)
                nc.vector.memset(vs[:, :, DK:DK + 1], 1.0)
                v_sb[kt] = vt

            yev_tiles = {}

            def oproj(g, m, evict_act=False):
                qh, hp, qc = g >> 2, (g >> 1) & 1, g & 1
                col = qh * 1024 + qc * 512
                ps = ps_d.tile([P, 512], f32, tag="d", name="psd")
                nc.tensor.matmul(
                    ps[:], lhsT=wo_sb[hp][:, m * P:(m + 1) * P],
                    rhs=out_sb[hp][:, col:col + 512], start=True, stop=True)
                if m % 2 == 0:
                    yev_tiles[g] = yev_pool.tile([P, 1024], bf16, tag="yev",
                                                 name="yev")
                ev = yev_tiles[g]
                half = ev[:, (m % 2) * 512:(m % 2 + 1) * 512]
                if evict_act:
                    nc.scalar.activation(half, ps[:], Copy)
                else:
                    nc.vector.tensor_copy(half, ps[:])
                if m % 2 == 1:
                    nc.gpsimd.dma_start(
                        y_dram[hp][(m - 1) * P:(m + 1) * P,
                                   col:col + 512].rearrange(
                            "(a p) f -> p a f", p=P),
                        ev[:].rearrange("p (a f) -> p a f", a=2))

            # ---------------- attention per-slot pieces ----------------
            sc_tiles = [None] * NSLOT
            et_tiles = [None] * NSLOT
            pt_tiles = [None] * NSLOT
            po_tiles = {}   # g -> [po_h2_0, po_h2_1]
            otmp_tiles = {}  # (g, h2) -> otmp

            def emit_scores(t):
                qh, hp, qc, kt = _slot(t)
                qcol = qh * 1024 + qc * 512
                ps = ps_s.tile([P, 1024], f32, tag="s", name="pss")
                for h2 in range(2):
                    r0 = h2 * DK
                    nc.tensor.matmul(
                        ps[:, h2 * 512:(h2 + 1) * 512],
                        lhsT=kt_sb[hp][r0:r0 + DK, kt * P:(kt + 1) * P],
                        rhs=qt_sb[hp][r0:r0 + DK, qcol:qcol + 512],
                        start=True, stop=True, tile_position=(r0, 0))
                sc_tiles[t] = ps

            def emit_exp(t):
                et = et_pool.tile([P, 1024], bf16, tag="et", name="et")
                nc.scalar.activation(et[:], sc_tiles[t][:], Exp, scale=0.125)
                et_tiles[t] = et
                sc_tiles[t] = None

            def emit_tt(t):
                qh, hp, qc, kt = _slot(t)
                pt = pt_pool.tile([P, 1024], bf16, tag="pt", name="pt")
                m_b = mask_sb[(qh, kt)][:, qc * 512:(qc + 1) * 512].rearrange(
                    "p (o f) -> p o f", o=1).broadcast_to([P, 2, 512])
                nc.vector.tensor_tensor(
                    pt[:].rearrange("p (o f) -> p o f", o=2),
                    et_tiles[t][:].rearrange("p (o f) -> p o f", o=2),
                    m_b, op=Mult)
                pt_tiles[t] = pt
                et_tiles[t] = None

            def emit_pv(t):
                qh, hp, qc, kt = _slot(t)
                g = t // 16
                if kt == 0:
                    po_tiles[g] = [
                        ps_o.tile([DK + 1, 512], f32, tag="po", name="po")
                        for _ in range(2)]
                po = po_tiles[g]
                pt = pt_tiles[t]
                for h2 in range(2):
                    h = 2 * hp + h2
                    nc.tensor.matmul(
                        po[h2][:],
                        lhsT=v_sb[kt][:, h * (DK + 1):(h + 1) * (DK + 1)],
                        rhs=pt[:, h2 * 512:(h2 + 1) * 512],
                        start=(kt == 0), stop=(kt == 15))
                pt_tiles[t] = None

            def emit_otmp(g):
                # copy PV accumulators out of PSUM (frees po slots)
                for h2 in range(2):
                    ot = otmp_pool.tile([DK + 1, 512], f32, tag="otmp",
                                        name="otmp")
                    nc.vector.tensor_copy(ot[:], po_tiles[g][h2][:])
                    otmp_tiles[(g, h2)] = ot
                del po_tiles[g]

            rr_tiles = {}
            bc_tiles = {}

            def emit_norm_a(g, h2):
                # ship the denominator row to DRAM and start the reshaped
                # read-back ([1,512] -> [128,4] so reciprocal uses lanes)
                ridx = g * 2 + h2
                ot = otmp_tiles[(g, h2)]
                nc.gpsimd.dma_start(rsum_dram[ridx:ridx + 1, :],
                                    ot[DK:DK + 1, :])
                rr = r_pool.tile([P, 4], f32, tag="r", name="rr")
                nc.gpsimd.dma_start(
                    rr[:],
                    rsum_dram[ridx:ridx + 1, :].rearrange(
                        "o (p f) -> (o p) f", p=P))
                rr_tiles[(g, h2)] = rr

            def emit_norm_b(g, h2):
                # reciprocal + partition-broadcast via DRAM
                ridx = g * 2 + h2
                rq = r_pool.tile([P, 4], f32, tag="r", name="rq")
                nc.vector.reciprocal(rq[:], rr_tiles[(g, h2)][:])
                rr_tiles[(g, h2)] = None
                nc.gpsimd.dma_start(
                    rrec_dram[ridx:ridx + 1, :].rearrange(
                        "o (p f) -> (o p) f", p=P),
                    rq[:])
                bc = bc_pool.tile([DK, 512], f32, tag="bc", name="bc")
                nc.gpsimd.dma_start(
                    bc[:], rrec_dram[ridx:ridx + 1, :].broadcast_to([DK, 512]))
                bc_tiles[(g, h2)] = bc

            def emit_norm_c(g, h2, mul_dve=False):
                qh, hp, qc = g >> 2, (g >> 1) & 1, g & 1
                qcol = qh * 1024 + qc * 512
                ot = otmp_tiles[(g, h2)]
                bc = bc_tiles[(g, h2)]
                dst = out_sb[hp][h2 * DK:(h2 + 1) * DK, qcol:qcol + 512]
                if mul_dve:
                    nc.vector.tensor_mul(dst, ot[0:DK, :], bc[:])
                else:
                    nc.gpsimd.tensor_mul(dst, ot[0:DK, :], bc[:])
                otmp_tiles[(g, h2)] = None
                bc_tiles[(g, h2)] = None

            # ---------------- schedules ----------------
            drips = {}   # iter -> list of closures

            def add(e, fn):
                drips.setdefault(e, []).append(fn)

            # V projections kt 1..15 just-in-time
            for kt in range(1, 16):
                add(kt - 1, lambda kt=kt: v_proj(kt))
            # K hp0 chunks c1..c3 (c0 in lead-in); scores(kt) needs
            # chunk kt//4 by iter kt-1
            for i, (c, h) in enumerate([(1, 0), (1, 1), (2, 0), (2, 1),
                                        (3, 0), (3, 1)]):
                e = [0, 1, 3, 4, 6, 7][i]
                add(e, lambda c=c, h=h: kq_half(kt_sb, "wk", "xk", 0, c, h))
            # Q hp0 chunk c1 (qh0/qc1, needed by iter 15)
            add(10, lambda: kq_half(qt_sb, "wq", "xq", 0, 1, 0))
            add(11, lambda: kq_half(qt_sb, "wq", "xq", 0, 1, 1))
            # K hp1 all chunks
            for i in range(8):
                c, h = i // 2, i % 2
                add(16 + i, lambda c=c, h=h: kq_half(kt_sb, "wk", "xk", 1, c, h))
            # Q hp1 c0, c1
            for i in range(4):
                c, h = i // 2, i % 2
                add(24 + i, lambda c=c, h=h: kq_half(qt_sb, "wq", "xq", 1, c, h))
            # Q qh1 chunks (c2, c3) for hp0 then hp1
            for i in range(4):
                c, h = 2 + i // 2, i % 2
                add(33 + i,
                    lambda c=c, h=h: kq_half(qt_sb, "wq", "xq", 0, c, h,
                                             act=False))
            for i in range(4):
                c, h = 2 + i // 2, i % 2
                add(49 + i,
                    lambda c=c, h=h: kq_half(qt_sb, "wq", "xq", 1, c, h,
                                             act=False))
            # normalization phases b (reciprocal) and c (scale) for g<=6;
            # phase a is emitted with the group's last PV at g*16+17
            for g in range(7):
                for h2 in range(2):
                    add(g * 16 + 19, lambda g=g, h2=h2: emit_norm_b(g, h2))
                    add(g * 16 + 21, lambda g=g, h2=h2: emit_norm_c(g, h2))
            # output projection drips for groups 0..6 (group 7 in tail);
            # every 3rd eviction goes to ACT to unload DVE
            for g in range(7):
                for m in range(8):
                    add(g * 16 + 26 + m, lambda g=g, m=m: oproj(g, m))

            # x chunk DMAs on the Sync queue (c0/xk1 in lead-in)
            add(0, lambda: dma_x("xv", xv, 1))
            add(1, lambda: dma_x("xk", xk, 2))
            add(3, lambda: dma_x("xv", xv, 2))
            add(4, lambda: dma_x("xk", xk, 3))
            add(6, lambda: dma_x("xv", xv, 3))
            add(7, lambda: dma_x("xq", xq, 1))
            add(28, lambda: dma_x("xq", xq, 2))
            add(29, lambda: dma_x("xq", xq, 3))

            # mask cast-DMAs (gpsimd queue, held back by the lead-in gate):
            # qh0 kt>=2 at kt-2; qh1 at 50+kt
            for kt in range(2, 16):
                add(kt - 2, lambda kt=kt: dma_mask(0, kt))
            for kt in range(16):
                add(50 + kt, lambda kt=kt: dma_mask(1, kt))

            # ---------------- lead-in ----------------
            # The DMA engines round-robin all queued descriptors, so
            # issue order alone gives no bandwidth priority.  Issue only
            # the critical path (wk, xk0, wq, xq0) first, then FENCE both
            # DMA streams with a tiny SBUF->DRAM read that depends on the
            # K projection eviction: later DMAs cannot issue (and steal
            # HBM bandwidth) until the critical path has landed.
            dma_w("wk", wk)
            dma_x("xk", xk, 0)
            dma_w("wq", wq)
            dma_x("xq", xq, 0)
            for h in range(2):
                kq_half(kt_sb, "wk", "xk", 0, 0, h)
            nc.sync.dma_start(fence_dram[0:1, :], kt_sb[0][0:1, 0:16])
            nc.gpsimd.dma_start(fence_dram[1:2, :], kt_sb[0][0:1, 0:16])
            dma_w("wv", wv)
            dma_x("xv", xv, 0)
            dma_x("xk", xk, 1)
            dma_wo()
            dma_mask(0, 0)
            dma_mask(0, 1)
            for h in range(2):
                kq_half(qt_sb, "wq", "xq", 0, 0, h)
            v_proj(0)

            # ---------------- main slot loop ----------------
            for e in range(-1, NSLOT + 2):
                if 0 <= e + 1 < NSLOT:
                    emit_scores(e + 1)
                if 0 <= e < NSLOT:
                    emit_exp(e)
                    emit_tt(e)
                if 0 <= e - 2 and e - 2 < NSLOT:
                    emit_pv(e - 2)
                    qh, hp, qc, kt = _slot(e - 2)
                    if kt == 15:
                        g = (e - 2) // 16
                        emit_otmp(g)
                        emit_norm_a(g, 0)
                        emit_norm_a(g, 1)
                for fn in drips.get(e, ()):
                    fn()

            # ---------------- tail: group 7 norm + output projection ---
            for h2 in range(2):
                emit_norm_b(7, h2)
            for h2 in range(2):
                emit_norm_c(7, h2, mul_dve=True)
            for m in range(8):
                oproj(7, m, evict_act=(m % 2 == 0))

    _split_multi_waits(nc)
    return nc


def _get_nc():
    if "nc" not in _CACHE:
        _CACHE["nc"] = _build_bass()
    return _CACHE["nc"]


def kernel(query, key, value, mask, w_q, w_k, w_v, w_o, **unused):
    nc = _get_nc()
    from concourse.bass_utils import run_bass_kernel_spmd

    in_maps = []
    for c in range(N_CORES):
        b = c // (N_CORES // B)
        hg = c % (N_CORES // B)
        e0 = hg * EPC
        in_maps.append({
            "xq_t": np.ascontiguousarray(query[b].T).astype(BF16),
            "xk_t": np.ascontiguousarray(key[b].T).astype(BF16),
            "xv_t": np.ascontiguousarray(value[b].T).astype(BF16),
            "mask_t": np.ascontiguousarray(mask[b].T).astype(
                ml_dtypes.float8_e4m3fn),
            "wq_t": np.ascontiguousarray(w_q[e0:e0 + EPC, :].T).astype(BF16),
            "wk_t": np.ascontiguousarray(w_k[e0:e0 + EPC, :].T).astype(BF16),
            "wv_t": np.ascontiguousarray(w_v[e0:e0 + EPC, :].T).astype(BF16),
            "wo_t": np.ascontiguousarray(w_o[:, e0:e0 + EPC].T).astype(BF16),
        })

    res = run_bass_kernel_spmd(nc, in_maps, core_ids=list(range(N_CORES)))
    _CACHE["last_results"] = res

    gpb = N_CORES // B
    out = np.empty((B, S, D), dtype=np.float32)
    for b in range(B):
        acc = None
        for c in range(b * gpb, (b + 1) * gpb):
            part = (res.results[c]["ya_t"].astype(np.float32)
                    + res.results[c]["yb_t"].astype(np.float32))
            acc = part if acc is None else acc + part
        out[b] = acc.T
    return out
